# revision 14
# baseline (speedup 1.0000x reference)
"""Trainium2 Bass kernel for nn_AbsGlobalHeadProbEncoder (MFVI message passing).

kernel(**inputs) takes the FULL inputs
    x       [4, 1024, 128] f32
    mask    [4, 1024]      i32   (all ones per the problem spec)
    ternary [128, 128, 8]  f32
    global_ [64, 128, 8]   f32
and returns the FULL output [4, 1024, 128] f32.

Sharding: 8 NeuronCores, one batch element per core pair (cores 2n and 2n+1
redundantly compute batch n with all 8 heads). A cross-core AllReduce variant
measured ~430 us/iteration of added latency - far more than the per-iteration
compute it saves - so full replication wins.

Per core and MFVI iteration (vs the f16 baseline):
- Scores F_H built with f16 matmuls into f32 PSUM, exp on the scalar engine
  (f16 out, bias -ln 16), normalized on DVE into fp8e4 scaled by 128 (the
  scale keeps softmax probabilities above fp8's subnormal floor; the final
  message is divided by 128 once, in fp32).
- msg_j and msg_i contractions run as fp8 DoubleRow matmuls (256-deep
  contraction per pass, 2x PE throughput).
- The [L, L] transpose for msg_i is done on fp8 data viewed as uint16 pairs:
  half the PE cycles, and the byte order directly produces the
  j-pair-interleaved layout DoubleRow wants (paired with a stride-2
  interleaved build of r = Qz @ T_h^T).
- Global-node messages amortized across head pairs (stacked 2G=128
  contraction).
- Heads are software-pipelined (head h+1's score build is emitted before
  head h's message matmuls) to keep the PE busy.
"""
import sys
import contextlib
import math

if '/opt/trn_rl_repo' not in sys.path:
    sys.path.insert(0, '/opt/trn_rl_repo')

import numpy as np
import concourse.bacc as bacc
import concourse.mybir as mybir
import concourse.tile as tile
from concourse.masks import make_identity

F32 = mybir.dt.float32
F16 = mybir.dt.float16
F8 = mybir.dt.float8e4
U16 = mybir.dt.uint16
AF = mybir.ActivationFunctionType
MUL = mybir.AluOpType.mult
ADD = mybir.AluOpType.add

B = 4
L = 1024
D = 128
H = 8
G = 64
IC = L // 128          # 8 i-chunks of 128 tokens
NG = L // 256          # 4 groups of 256 tokens (DoubleRow contraction)
N_CORES = 8
EBIAS = float(-math.log(16.0))   # exp bias: keeps f16 e in range
ESCALE = 128.0                   # prob scale: keeps fp8 e above subnormals

_CACHE = {}


def build_kernel(n_iter=4, num_devices=8, mode='dr'):
    nc = bacc.Bacc("TRN2", target_bir_lowering=False, debug=False,
                   num_devices=num_devices)

    xT = nc.declare_dram_parameter("xT", [D, L], F32, isOutput=False)
    ta = nc.declare_dram_parameter("ta", [D, H * D], F16, isOutput=False)
    tb = nc.declare_dram_parameter("tb", [D, H * D], F16, isOutput=False)
    glT = nc.declare_dram_parameter("glT", [D, H * G], F16, isOutput=False)
    gl2 = nc.declare_dram_parameter("gl2", [2 * G, (H // 2) * D], F16, isOutput=False)
    y = nc.declare_dram_parameter("y", [L, D], F32, isOutput=True)

    with tile.TileContext(nc) as tc:
        with contextlib.ExitStack() as ctx:
            singles = ctx.enter_context(tc.tile_pool(name="singles", bufs=1))
            sb = ctx.enter_context(tc.tile_pool(name="sb", bufs=2))
            hp = ctx.enter_context(tc.tile_pool(name="hp", bufs=2))
            # PSUM budget (8 banks): msgp 2 + fh 2x2 + tp 2x1 = 8
            fh = ctx.enter_context(tc.tile_pool(name="fh", bufs=2, space="PSUM"))
            tp = ctx.enter_context(tc.tile_pool(name="tp", bufs=2, space="PSUM"))
            msgp = ctx.enter_context(tc.tile_pool(name="msgp", bufs=1, space="PSUM"))

            # ---- persistent SBUF state ----
            unaryT = singles.tile([D, L], F32)
            nc.sync.dma_start(unaryT[:], xT[:])
            ta_sb = singles.tile([D, H * D], F16)
            nc.sync.dma_start(ta_sb[:], ta[:])
            tb_sb = singles.tile([D, H * D], F16)
            nc.sync.dma_start(tb_sb[:], tb[:])
            glT_sb = singles.tile([D, H * G], F16)
            nc.sync.dma_start(glT_sb[:], glT[:])
            gl2_sb = singles.tile([2 * G, (H // 2) * D], F16)
            nc.sync.dma_start(gl2_sb[:], gl2[:])
            id16 = singles.tile([128, 128], F16)
            make_identity(nc, id16[:])
            id32 = singles.tile([128, 128], F32)
            make_identity(nc, id32[:])
            id8 = singles.tile([128, 128], F8)
            nc.vector.tensor_copy(id8[:], id16[:])  # f16 1.0 -> f8 1.0

            qzT = singles.tile([D, L], F16)
            fzT = singles.tile([D, L], F32)
            s8 = singles.tile([128, IC * H * 128], F8)      # [i, (c,h,b)]
            r2 = singles.tile([128, 2 * NG * H * 128], F8)  # [jp, (g,t,h,a)]
            eg = singles.tile([128, IC * H * G], F16)       # [i, (ic,h,g)]
            eg_sums = singles.tile([128, IC * H], F32)      # [i, (ic,h)]
            rr_all = singles.tile([128, IC * H], F32)       # [i, (ic,h)] scaled
            egT2 = singles.tile([128, (H // 2) * L], F16)   # [(hh,g), (k,i)]

            s8_r = s8.rearrange("p (c h b) -> p c h b", c=IC, h=H)
            r2_r = r2.rearrange("p (gt h a) -> p gt h a", gt=2 * NG, h=H)
            eg_r = eg.rearrange("p (s h g) -> p s h g", s=IC, h=H)
            eg_sums_r = eg_sums.rearrange("p (s h) -> p s h", h=H)
            rr_all_r = rr_all.rearrange("p (s h) -> p s h", h=H)
            # qzT tokens split (g:4, j:128, t:2): token = g*256 + j*2 + t
            qzT_gt = qzT.rearrange("p (g j t) -> p g t j", g=NG, t=2)

            def z_tail(fzT_src, last=False):
                """qzT <- softmax_D(fzT_src^T)^T ; if last: y <- fzT_src^T."""
                if last:
                    out_sb = sb.tile([128, L], F32, tag="zout")
                    for ic in range(IC):
                        fz_ps = fh.tile([128, 128], F32, tag="fh_ps")
                        nc.tensor.transpose(fz_ps[:], fzT_src[:, ic * 128:(ic + 1) * 128], id32[:])
                        nc.vector.tensor_copy(out_sb[:, ic * 128:(ic + 1) * 128], fz_ps[:])
                        nc.sync.dma_start(y[ic * 128:(ic + 1) * 128, :],
                                          out_sb[:, ic * 128:(ic + 1) * 128])
                    return
                ez = sb.tile([128, L], F32, tag="ez")
                sums = sb.tile([128, IC], F32, tag="zsums")
                for half in range(2):
                    fz_ps = fh.tile([128, 512], F32, tag="fh_ps")
                    for k in range(4):
                        ic = half * 4 + k
                        nc.tensor.transpose(fz_ps[:, k * 128:(k + 1) * 128],
                                            fzT_src[:, ic * 128:(ic + 1) * 128], id32[:])
                    for k in range(4):
                        ic = half * 4 + k
                        nc.scalar.activation(ez[:, ic * 128:(ic + 1) * 128],
                                             fz_ps[:, k * 128:(k + 1) * 128], AF.Exp,
                                             accum_out=sums[:, ic:ic + 1])
                rz = sb.tile([128, IC], F32, tag="zrz")
                nc.vector.reciprocal(rz[:], sums[:])
                qz_sc = sb.tile([128, L], F16, tag="qzsc")
                for ic in range(IC):
                    cs = slice(ic * 128, (ic + 1) * 128)
                    nc.gpsimd.tensor_scalar_mul(qz_sc[:, cs], ez[:, cs], rz[:, ic:ic + 1])
                qzT_ps = tp.tile([128, L], F16, tag="tp_ps")
                for ic in range(IC):
                    cs = slice(ic * 128, (ic + 1) * 128)
                    nc.tensor.transpose(qzT_ps[:, cs], qz_sc[:, cs], id16[:])
                nc.vector.tensor_copy(qzT[:], qzT_ps[:])

            z_tail(unaryT)

            for it in range(n_iter):
                # ---------- phase A: shared across heads ----------
                # s-build: s8[i, (c,h,b)] = (Qz T_h)[i, b], fp8
                for c in range(IC):
                    cs = slice(c * 128, (c + 1) * 128)
                    s_ps = fh.tile([128, H * 128], F32, tag="fh_ps")
                    for half in range(2):
                        nh = slice(half * 512, (half + 1) * 512)
                        nc.tensor.matmul(s_ps[:, nh], qzT[:, cs], ta_sb[:, nh])
                    nc.vector.tensor_copy(s8[:, c * 1024:(c + 1) * 1024], s_ps[:])
                # r-build: r2[j, (c,h,a)] = (Qz T_h^T)[j, a]
                for g in range(NG):
                    for t in range(2):
                        r_ps = fh.tile([128, H * 128], F32, tag="fh_ps")
                        if mode == 'dr':
                            lhs = qzT_gt[:, g, t, :]
                        else:
                            c = g * 2 + t
                            lhs = qzT[:, c * 128:(c + 1) * 128]
                        for half in range(2):
                            nh = slice(half * 512, (half + 1) * 512)
                            nc.tensor.matmul(r_ps[:, nh], lhs, tb_sb[:, nh])
                        o = (g * 2 + t) * 1024
                        if (g * 2 + t) % 2 == 0:
                            nc.vector.tensor_copy(r2[:, o:o + 1024], r_ps[:])
                        else:
                            nc.scalar.copy(r2[:, o:o + 1024], r_ps[:])
                # hg-build + exp
                for ic in range(IC):
                    cs = slice(ic * 128, (ic + 1) * 128)
                    hg_ps = fh.tile([128, H * G], F32, tag="fh_ps")
                    nc.tensor.matmul(hg_ps[:], qzT[:, cs], glT_sb[:])
                    nc.scalar.activation(eg[:, ic * 512:(ic + 1) * 512], hg_ps[:],
                                         AF.Exp)
                nc.vector.reduce_sum(eg_sums[:],
                                     eg.rearrange("p (sh g) -> p sh g", g=G),
                                     axis=mybir.AxisListType.X)

                msg_ps = msgp.tile([128, L], F32, tag="msg_ps")

                # ---------- phase B: per head, software-pipelined ----------
                sts = [None] * H
                e16s = [None] * H
                e8s = [None] * H
                sums_t = [None] * H

                def emit_front(h):
                    """st + F-build + exp for head h."""
                    hs = slice(h * 128, (h + 1) * 128)
                    st_ps = fh.tile([128, L], F32, tag="fh_ps")
                    for half in range(2):
                        nc.tensor.matmul(st_ps[:, half * 512:(half + 1) * 512],
                                         ta_sb[:, hs], qzT[:, half * 512:(half + 1) * 512])
                    st = hp.tile([128, L], F16, tag="st_sb")
                    nc.vector.tensor_copy(st[:], st_ps[:])
                    e16 = hp.tile([128, IC * L], F16, tag="e16")
                    sums = hp.tile([128, IC], F32, tag="hsums")
                    for ic in range(IC):
                        fh_ps = fh.tile([128, L], F32, tag="fh_ps")
                        for half in range(2):
                            nc.tensor.matmul(fh_ps[:, half * 512:(half + 1) * 512],
                                             st[:, ic * 128:(ic + 1) * 128],
                                             qzT[:, half * 512:(half + 1) * 512])
                        nc.scalar.activation(e16[:, ic * L:(ic + 1) * L], fh_ps[:],
                                             AF.Exp,
                                             accum_out=sums[:, ic:ic + 1])
                    sts[h], e16s[h], sums_t[h] = st, e16, sums

                def emit_back(h):
                    """normalize + msg_j + transpose + msg_i for head h."""
                    e16, sums = e16s[h], sums_t[h]
                    tot = hp.tile([128, IC], F32, tag="htot")
                    rr = hp.tile([128, IC], F32, tag="hrr")
                    nc.vector.tensor_add(tot[:], sums[:], eg_sums_r[:, :, h])
                    nc.vector.reciprocal(rr[:], tot[:])
                    # rr2 = ESCALE / tot, stored into rr_all for the eg tail
                    nc.vector.tensor_scalar_mul(rr_all_r[:, :, h], rr[:], ESCALE)
                    e8t = hp.tile([128, IC * L], F8, tag="e8")
                    for ic in range(IC):
                        es = slice(ic * L, (ic + 1) * L)
                        eng = nc.gpsimd if ic >= 6 else nc.vector
                        eng.tensor_scalar_mul(e8t[:, es], e16[:, es],
                                              rr_all_r[:, ic:ic + 1, h])
                    e8_c = e8t.rearrange("p (c j) -> p c j", c=IC)
                    # msg_j: out[b, j] += sum_i s8[i, b] * e8[i, j]
                    if mode == 'dr':
                        for g in range(NG):
                            lhs = s8_r[:, 2 * g:2 * g + 2, h, :]
                            for half in range(2):
                                nc.tensor.matmul(
                                    msg_ps[:, half * 512:(half + 1) * 512],
                                    lhs, e8_c[:, 2 * g:2 * g + 2, half * 512:(half + 1) * 512],
                                    start=(h == 0 and g == 0), stop=False,
                                    perf_mode=mybir.MatmulPerfMode.DoubleRow)
                    else:
                        for c in range(IC):
                            for half in range(2):
                                nc.tensor.matmul(
                                    msg_ps[:, half * 512:(half + 1) * 512],
                                    s8_r[:, c, h, :],
                                    e8_c[:, c, half * 512:(half + 1) * 512],
                                    start=(h == 0 and c == 0 and False) or (h == 0 and c == 0),
                                    stop=False)
                    if mode == 'dr':
                        # transpose e8 as u16 pairs -> et2[jp, (g, i)]
                        e8_u = e8t[:].bitcast(F16)  # [128, IC*512]
                        et2 = hp.tile([128, NG * L], F16, tag="et2")
                        for g in range(NG):
                            t_ps = tp.tile([128, L], F16, tag="tp_ps")
                            for ic in range(IC):
                                nc.tensor.transpose(
                                    t_ps[:, ic * 128:(ic + 1) * 128],
                                    e8_u[:, ic * 512 + g * 128: ic * 512 + (g + 1) * 128],
                                    id16[:])
                            nc.vector.tensor_copy(et2[:, g * L:(g + 1) * L], t_ps[:])
                        # msg_i: out[a, i] += sum_j r2[jp, t, a] * et[j, i]
                        et2_f8 = et2[:].bitcast(F8).rearrange("p (g i t) -> p g t i",
                                                              g=NG, t=2)
                        for g in range(NG):
                            lhs = r2_r[:, 2 * g:2 * g + 2, h, :]
                            for half in range(2):
                                nc.tensor.matmul(
                                    msg_ps[:, half * 512:(half + 1) * 512],
                                    lhs, et2_f8[:, g, :, half * 512:(half + 1) * 512],
                                    start=False, stop=False,
                                    perf_mode=mybir.MatmulPerfMode.DoubleRow)
                    else:
                        # unpacked f8 transposes -> et2[j, (jc, i)]
                        et2 = hp.tile([128, IC * L], F8, tag="et2")
                        for jc in range(IC):
                            t_ps = tp.tile([128, L], F8, tag="tp_ps")
                            for ic in range(IC):
                                nc.tensor.transpose(
                                    t_ps[:, ic * 128:(ic + 1) * 128],
                                    e8_c[:, ic, jc * 128:(jc + 1) * 128],
                                    id8[:])
                            nc.vector.tensor_copy(et2[:, jc * L:(jc + 1) * L], t_ps[:])
                        et2_c = et2.rearrange("p (c i) -> p c i", c=IC)
                        for jc in range(IC):
                            for half in range(2):
                                nc.tensor.matmul(
                                    msg_ps[:, half * 512:(half + 1) * 512],
                                    r2_r[:, jc, h, :],
                                    et2_c[:, jc, half * 512:(half + 1) * 512],
                                    start=False, stop=False)
                    sts[h] = e16s[h] = sums_t[h] = None

                emit_front(0)
                for h in range(H):
                    if h + 1 < H:
                        emit_front(h + 1)
                    emit_back(h)

                # ---------- eg tail: normalize + pair transposes + msg_g ----------
                for h in range(H):
                    for ic in range(IC):
                        nc.gpsimd.tensor_scalar_mul(eg_r[:, ic, h, :], eg_r[:, ic, h, :],
                                                    rr_all_r[:, ic:ic + 1, h])
                for k in range(H // 2):
                    tg_ps = tp.tile([128, L], F16, tag="tp_ps")
                    for ic in range(IC):
                        nc.tensor.transpose(tg_ps[:, ic * 128:(ic + 1) * 128],
                                            eg[:, ic * 512 + k * 128: ic * 512 + (k + 1) * 128],
                                            id16[:])
                    nc.vector.tensor_copy(egT2[:, k * L:(k + 1) * L], tg_ps[:])
                for k in range(H // 2):
                    for half in range(2):
                        nc.tensor.matmul(
                            msg_ps[:, half * 512:(half + 1) * 512],
                            gl2_sb[:, k * 128:(k + 1) * 128],
                            egT2[:, k * L + half * 512: k * L + (half + 1) * 512],
                            start=False, stop=(k == H // 2 - 1))

                # ---------- phase C: combine + Z update ----------
                nc.vector.scalar_tensor_tensor(fzT[:], msg_ps[:], 1.0 / ESCALE,
                                               unaryT[:], MUL, ADD)
                z_tail(fzT, last=(it == n_iter - 1))

    nc.compile()
    return nc


class _Runner:
    """Keeps the jitted SPMD executable alive across kernel() calls."""

    def __init__(self, nc):
        import jax
        from jax.sharding import Mesh, PartitionSpec
        from jax.experimental.shard_map import shard_map
        from concourse.bass2jax import (_bass_exec_p, install_neuronx_cc_hook,
                                        partition_id_tensor)
        install_neuronx_cc_hook()
        self.jax = jax
        in_names, out_names, out_avals, zero_outs = [], [], [], []
        partition_name = nc.partition_id_tensor.name if nc.partition_id_tensor else None
        for alloc in nc.m.functions[0].allocations:
            if not isinstance(alloc, mybir.MemoryLocationSet):
                continue
            name = alloc.memorylocations[0].name
            if alloc.kind == "ExternalInput":
                if name != partition_name:
                    in_names.append(name)
            elif alloc.kind == "ExternalOutput":
                out_names.append(name)
                shape = tuple(alloc.tensor_shape)
                dtype = mybir.dt.np(alloc.dtype)
                out_avals.append(jax.core.ShapedArray(shape, dtype))
                zero_outs.append(np.zeros(shape, dtype))
        self.in_names, self.out_names = in_names, out_names
        self.out_avals, self.zero_outs = out_avals, zero_outs
        all_in_names = list(in_names) + list(out_names)
        if partition_name is not None:
            all_in_names.append(partition_name)

        def _body(*args):
            operands = list(args)
            if partition_name is not None:
                operands.append(partition_id_tensor())
            outs = _bass_exec_p.bind(
                *operands,
                out_avals=tuple(out_avals),
                in_names=tuple(all_in_names),
                out_names=tuple(out_names),
                lowering_input_output_aliases=(),
                sim_require_finite=True,
                sim_require_nnan=True,
                nc=nc,
            )
            return tuple(outs)

        devices = jax.devices()[:N_CORES]
        mesh = Mesh(np.asarray(devices), ("core",))
        n_params = len(in_names)
        in_specs = (PartitionSpec("core"),) * (n_params + len(out_names))
        out_specs = (PartitionSpec("core"),) * len(out_names)
        self.fn = jax.jit(shard_map(_body, mesh=mesh, in_specs=in_specs,
                                    out_specs=out_specs, check_rep=False),
                          keep_unused=True)

    def __call__(self, in_maps):
        jax = self.jax
        concat_in = [
            np.concatenate([np.asarray(in_maps[c][name]) for c in range(N_CORES)], axis=0)
            for name in self.in_names
        ]
        concat_zeros = [np.zeros((N_CORES * z.shape[0], *z.shape[1:]), z.dtype)
                        for z in self.zero_outs]
        outs = self.fn(*concat_in, *concat_zeros)
        jax.block_until_ready(outs)
        return [
            {name: np.asarray(outs[i]).reshape(N_CORES, *self.out_avals[i].shape)[c]
             for i, name in enumerate(self.out_names)}
            for c in range(N_CORES)
        ]


def make_core_inputs(x, ternary, global_, core, hpc=None):
    n = core // 2
    t = ternary.astype(np.float32)
    g = global_.astype(np.float32)
    # gl2[(hh, g), (k, a)] = global_[g, a, 2k+hh]
    gg = g.transpose(2, 0, 1).reshape(H // 2, 2, G, D)       # [k, hh, G, D]
    gl2 = gg.transpose(1, 2, 0, 3).reshape(2 * G, (H // 2) * D)
    return {
        "xT": np.ascontiguousarray(x[n].T.astype(np.float32)),
        "ta": np.ascontiguousarray(t.transpose(0, 2, 1).reshape(D, H * D).astype(np.float16)),
        "tb": np.ascontiguousarray(t.transpose(1, 2, 0).reshape(D, H * D).astype(np.float16)),
        "glT": np.ascontiguousarray(g.transpose(1, 2, 0).reshape(D, H * G).astype(np.float16)),
        "gl2": np.ascontiguousarray(gl2.astype(np.float16)),
    }


def get_runner(n_iter=4):
    key = ("runner", n_iter)
    if key not in _CACHE:
        nc = build_kernel(n_iter=n_iter, num_devices=N_CORES)
        _CACHE[key] = _Runner(nc)
    return _CACHE[key]


def kernel(x, mask, ternary, global_):
    x = np.asarray(x, dtype=np.float32)
    mask = np.asarray(mask)
    ternary = np.asarray(ternary, dtype=np.float32)
    global_ = np.asarray(global_, dtype=np.float32)

    run = get_runner(4)
    in_maps = [make_core_inputs(x, ternary, global_, c) for c in range(N_CORES)]
    res = run(in_maps)
    out = np.stack([res[2 * n]["y"] for n in range(B)])
    out = np.where((mask != 0)[..., None], out, np.float32(0.0)).astype(np.float32)
    return out


# revision 15
# speedup vs baseline: 2.8656x; 2.8656x over previous
"""Trainium2 Bass kernel for nn_AbsGlobalHeadProbEncoder (MFVI message passing).

kernel(**inputs) takes the FULL inputs
    x       [4, 1024, 128] f32
    mask    [4, 1024]      i32   (all ones per the problem spec)
    ternary [128, 128, 8]  f32
    global_ [64, 128, 8]   f32
and returns the FULL output [4, 1024, 128] f32.

Sharding: 8 NeuronCores, one batch element per core pair (cores 2n and 2n+1
redundantly compute batch n with all 8 heads). A cross-core AllReduce variant
was measured at ~2.2 ms per 512 KB pairwise all-reduce on this stack - far
more than the ~0.15 ms of per-iteration compute it would save - so full
replication wins.

Per core and MFVI iteration: scores F_H = [H, L, L+G] are built chunk-wise
with fp16 matmuls into fp32 PSUM, softmax'd with a fused exp+rowsum on the
scalar engine, normalized by a per-partition DVE scale, transposed on the PE
(fp16, packed PSUM banks) for the msg_i contraction, and all three messages
accumulate into one shared [D, L] fp32 PSUM region.
"""
import sys
import contextlib

if '/opt/trn_rl_repo' not in sys.path:
    sys.path.insert(0, '/opt/trn_rl_repo')

import numpy as np
import concourse.bacc as bacc
import concourse.mybir as mybir
import concourse.tile as tile
from concourse.masks import make_identity

F32 = mybir.dt.float32
F16 = mybir.dt.float16
AF = mybir.ActivationFunctionType

B = 4
L = 1024
D = 128
H = 8
G = 64
IC = L // 128
N_CORES = 8

_CACHE = {}


def build_kernel(n_iter=4, num_devices=8, groups=None, hpc=4, sb_bufs=2, use_cc=True):
    if groups is None:
        groups = [[2 * i, 2 * i + 1] for i in range(num_devices // 2)]
    nc = bacc.Bacc("TRN2", target_bir_lowering=False, debug=False,
                   num_devices=num_devices)

    xT = nc.declare_dram_parameter("xT", [D, L], F32, isOutput=False)
    tern_a = nc.declare_dram_parameter("tern_a", [D, hpc * D], F16, isOutput=False)
    tern_b = nc.declare_dram_parameter("tern_b", [D, hpc * D], F16, isOutput=False)
    glT = nc.declare_dram_parameter("glT", [D, hpc * G], F16, isOutput=False)
    gl = nc.declare_dram_parameter("gl", [G, hpc * D], F16, isOutput=False)
    y = nc.declare_dram_parameter("y", [L, D], F32, isOutput=True)

    with tile.TileContext(nc) as tc:
        with contextlib.ExitStack() as ctx:
            singles = ctx.enter_context(tc.tile_pool(name="singles", bufs=1))
            sb = ctx.enter_context(tc.tile_pool(name="sb", bufs=sb_bufs))
            hp = ctx.enter_context(tc.tile_pool(name="hp", bufs=2))
            # PSUM budget (8 banks): msgp 2 + fh 2x2 + aux 2x1 = 8
            aux = ctx.enter_context(tc.tile_pool(name="aux", bufs=2, space="PSUM"))
            fh = ctx.enter_context(tc.tile_pool(name="fh", bufs=2, space="PSUM"))
            msgp = ctx.enter_context(tc.tile_pool(name="msgp", bufs=1, space="PSUM"))
            dram = ctx.enter_context(tc.tile_pool(name="dram", bufs=2, space="DRAM"))

            # ---- persistent SBUF state ----
            unaryT = singles.tile([D, L], F32)
            nc.sync.dma_start(unaryT[:], xT[:])
            ta_sb = singles.tile([D, hpc * D], F16)
            nc.sync.dma_start(ta_sb[:], tern_a[:])
            tb_sb = singles.tile([D, hpc * D], F16)
            nc.sync.dma_start(tb_sb[:], tern_b[:])
            glT_sb = singles.tile([D, hpc * G], F16)
            nc.sync.dma_start(glT_sb[:], glT[:])
            gl_sb = singles.tile([G, hpc * D], F16)
            nc.sync.dma_start(gl_sb[:], gl[:])
            id16 = singles.tile([128, 128], F16)
            make_identity(nc, id16[:])
            id32 = singles.tile([128, 128], F32)
            make_identity(nc, id32[:])

            qzT = singles.tile([D, L], F16)
            fzT = singles.tile([D, L], F32)

            def z_tail(fzT_src, last=False):
                """qzT <- softmax_D(fzT_src^T)^T ; if last: y <- fzT_src^T."""
                if last:
                    out_sb = sb.tile([128, L], F32, tag="zout")
                    for ic in range(IC):
                        fz_ps = fh.tile([128, 128], F32, tag="fh_ps")
                        nc.tensor.transpose(fz_ps[:], fzT_src[:, ic * 128:(ic + 1) * 128], id32[:])
                        nc.vector.tensor_copy(out_sb[:, ic * 128:(ic + 1) * 128], fz_ps[:])
                        nc.sync.dma_start(y[ic * 128:(ic + 1) * 128, :],
                                          out_sb[:, ic * 128:(ic + 1) * 128])
                    return
                ez = sb.tile([128, L], F32, tag="ez")
                sums = sb.tile([128, IC], F32, tag="zsums")
                for ic in range(IC):
                    cs = slice(ic * 128, (ic + 1) * 128)
                    fz_ps = fh.tile([128, 128], F32, tag="fh_ps")
                    nc.tensor.transpose(fz_ps[:], fzT_src[:, cs], id32[:])
                    nc.scalar.activation(ez[:, cs], fz_ps[:], AF.Exp,
                                         accum_out=sums[:, ic:ic + 1])
                rz = sb.tile([128, IC], F32, tag="zrz")
                nc.vector.reciprocal(rz[:], sums[:])
                qz_sc = sb.tile([128, L], F16, tag="qzsc")
                for ic in range(IC):
                    cs = slice(ic * 128, (ic + 1) * 128)
                    nc.vector.tensor_scalar_mul(qz_sc[:, cs], ez[:, cs], rz[:, ic:ic + 1])
                qzT_ps = fh.tile([128, L], F16, tag="fh_ps")
                for ic in range(IC):
                    cs = slice(ic * 128, (ic + 1) * 128)
                    nc.tensor.transpose(qzT_ps[:, cs], qz_sc[:, cs], id16[:])
                nc.vector.tensor_copy(qzT[:], qzT_ps[:])

            z_tail(unaryT)

            for it in range(n_iter):
                # ---------- phase A: shared across this core's heads ----------
                s_sb = sb.tile([128, IC * hpc * 128], F16, tag="s_sb")
                r_sb = sb.tile([128, IC * hpc * 128], F16, tag="r_sb")
                for c in range(IC):
                    cs = slice(c * 128, (c + 1) * 128)
                    os_ = slice(c * hpc * 128, (c + 1) * hpc * 128)
                    s_ps = fh.tile([128, hpc * 128], F32, tag="fh_ps")
                    for half in range(max(1, hpc * 128 // 512)):
                        nh = slice(half * 512, min((half + 1) * 512, hpc * 128))
                        nc.tensor.matmul(s_ps[:, nh], qzT[:, cs], ta_sb[:, nh])
                    nc.scalar.copy(s_sb[:, os_], s_ps[:])
                    r_ps = fh.tile([128, hpc * 128], F32, tag="fh_ps")
                    for half in range(max(1, hpc * 128 // 512)):
                        nh = slice(half * 512, min((half + 1) * 512, hpc * 128))
                        nc.tensor.matmul(r_ps[:, nh], qzT[:, cs], tb_sb[:, nh])
                    nc.scalar.copy(r_sb[:, os_], r_ps[:])
                eg_sb = sb.tile([128, IC * hpc * G], F16, tag="eg_sb")
                ics_per_bank = max(1, 512 // (hpc * G))
                for ic2 in range(IC // ics_per_bank):
                    hg_ps = fh.tile([128, ics_per_bank * hpc * G], F32, tag="fh_ps")
                    for k in range(ics_per_bank):
                        ic = ics_per_bank * ic2 + k
                        nc.tensor.matmul(hg_ps[:, k * hpc * G:(k + 1) * hpc * G],
                                         qzT[:, ic * 128:(ic + 1) * 128], glT_sb[:])
                    nc.scalar.activation(
                        eg_sb[:, ic2 * ics_per_bank * hpc * G:(ic2 + 1) * ics_per_bank * hpc * G],
                        hg_ps[:], AF.Exp)
                eg_sums = sb.tile([128, IC * hpc], F32, tag="eg_sums")
                nc.vector.reduce_sum(eg_sums[:],
                                     eg_sb.rearrange("p (s g) -> p s g", g=G),
                                     axis=mybir.AxisListType.X)

                msg_ps = msgp.tile([128, L], F32, tag="msg_ps")

                # ---------- phase B: per head ----------
                for h in range(hpc):
                    hs = slice(h * 128, (h + 1) * 128)
                    st_ps = fh.tile([128, L], F32, tag="fh_ps")
                    for half in range(2):
                        nc.tensor.matmul(st_ps[:, half * 512:(half + 1) * 512],
                                         ta_sb[:, hs], qzT[:, half * 512:(half + 1) * 512])
                    st_sb = hp.tile([128, L], F16, tag="st_sb")
                    nc.scalar.copy(st_sb[:], st_ps[:])

                    e_big = hp.tile([128, IC * L], F16, tag="e_big")
                    et_big = hp.tile([128, IC * L], F16, tag="et_big")
                    sums = hp.tile([128, IC], F32, tag="hsums")
                    tot = hp.tile([128, IC], F32, tag="htot")
                    rr = hp.tile([128, IC], F32, tag="hr")
                    for ic in range(IC):
                        fh_ps = fh.tile([128, L], F32, tag="fh_ps")
                        for half in range(2):
                            nc.tensor.matmul(fh_ps[:, half * 512:(half + 1) * 512],
                                             st_sb[:, ic * 128:(ic + 1) * 128],
                                             qzT[:, half * 512:(half + 1) * 512])
                        nc.scalar.activation(e_big[:, ic * L:(ic + 1) * L], fh_ps[:],
                                             AF.Exp, accum_out=sums[:, ic:ic + 1])
                    eg_h_sums = eg_sums.rearrange("p (s h) -> p s h", h=hpc)[:, :, h]
                    nc.vector.tensor_add(tot[:], sums[:], eg_h_sums)
                    nc.vector.reciprocal(rr[:], tot[:])
                    for ic in range(IC):
                        es = slice(ic * L, (ic + 1) * L)
                        nc.vector.tensor_scalar_mul(e_big[:, es], e_big[:, es],
                                                    rr[:, ic:ic + 1])
                        for half in range(2):
                            nc.tensor.matmul(
                                msg_ps[:, half * 512:(half + 1) * 512],
                                s_sb[:, (ic * hpc + h) * 128:(ic * hpc + h + 1) * 128],
                                e_big[:, ic * L + half * 512: ic * L + (half + 1) * 512],
                                start=(h == 0 and ic == 0), stop=False)
                        t_ps = aux.tile([128, L], F16, tag="aux_ps")
                        for jc in range(IC):
                            nc.tensor.transpose(t_ps[:, jc * 128:(jc + 1) * 128],
                                                e_big[:, ic * L + jc * 128: ic * L + (jc + 1) * 128],
                                                id16[:])
                        cp_eng = nc.vector.tensor_copy
                        cp_eng(
                            et_big.rearrange("p (jc i) -> p jc i", jc=IC)[:, :, ic * 128:(ic + 1) * 128],
                            t_ps.rearrange("p (jc i) -> p jc i", jc=IC))
                    for jc in range(IC):
                        for half in range(2):
                            nc.tensor.matmul(
                                msg_ps[:, half * 512:(half + 1) * 512],
                                r_sb[:, (jc * hpc + h) * 128:(jc * hpc + h + 1) * 128],
                                et_big[:, jc * L + half * 512: jc * L + (half + 1) * 512],
                                start=False, stop=False)
                    egT_ps = aux.tile([64, IC * 128], F16, tag="aux_ps")
                    for ic in range(IC):
                        col = (ic * hpc + h) * G
                        nc.vector.tensor_scalar_mul(eg_sb[:, col:col + G],
                                                    eg_sb[:, col:col + G], rr[:, ic:ic + 1])
                        nc.tensor.transpose(egT_ps[:, ic * 128:(ic + 1) * 128],
                                            eg_sb[:, col:col + G], id16[:])
                    egT_sb = hp.tile([64, IC * 128], F16, tag="egT_sb")
                    nc.scalar.copy(egT_sb[:], egT_ps[:])
                    for half in range(2):
                        nc.tensor.matmul(msg_ps[:, half * 512:(half + 1) * 512],
                                         gl_sb[:, hs],
                                         egT_sb[:, half * 512:(half + 1) * 512],
                                         start=False, stop=(h == hpc - 1))

                # ---------- phase C: all-reduce + Z update ----------
                if use_cc:
                    msg_sb = sb.tile([128, L], F32, tag="msg_sb")
                    nc.vector.tensor_copy(msg_sb[:], msg_ps[:])
                    bi = dram.tile([128, L], F32, tag="cc_in")
                    bo = dram.tile([128, L], F32, tag="cc_out")
                    nc.sync.dma_start(bi[:], msg_sb[:])
                    nc.gpsimd.collective_compute(
                        "AllReduce", mybir.AluOpType.add,
                        replica_groups=groups,
                        ins=[bi.opt()], outs=[bo.opt()])
                    msg_red = sb.tile([128, L], F32, tag="msg_red")
                    nc.sync.dma_start(msg_red[:], bo[:])
                    nc.vector.tensor_add(fzT[:], msg_red[:], unaryT[:])
                else:
                    nc.vector.tensor_add(fzT[:], msg_ps[:], unaryT[:])
                z_tail(fzT, last=(it == n_iter - 1))

    nc.compile()
    return nc

class _Runner:
    """Keeps the jitted SPMD executable alive across kernel() calls."""

    def __init__(self, nc):
        import jax
        from jax.sharding import Mesh, PartitionSpec
        from jax.experimental.shard_map import shard_map
        from concourse.bass2jax import (_bass_exec_p, install_neuronx_cc_hook,
                                        partition_id_tensor)
        install_neuronx_cc_hook()
        self.jax = jax
        in_names, out_names, out_avals, zero_outs = [], [], [], []
        partition_name = nc.partition_id_tensor.name if nc.partition_id_tensor else None
        for alloc in nc.m.functions[0].allocations:
            if not isinstance(alloc, mybir.MemoryLocationSet):
                continue
            name = alloc.memorylocations[0].name
            if alloc.kind == "ExternalInput":
                if name != partition_name:
                    in_names.append(name)
            elif alloc.kind == "ExternalOutput":
                out_names.append(name)
                shape = tuple(alloc.tensor_shape)
                dtype = mybir.dt.np(alloc.dtype)
                out_avals.append(jax.core.ShapedArray(shape, dtype))
                zero_outs.append(np.zeros(shape, dtype))
        self.in_names, self.out_names = in_names, out_names
        self.out_avals, self.zero_outs = out_avals, zero_outs
        all_in_names = list(in_names) + list(out_names)
        if partition_name is not None:
            all_in_names.append(partition_name)

        def _body(*args):
            operands = list(args)
            if partition_name is not None:
                operands.append(partition_id_tensor())
            outs = _bass_exec_p.bind(
                *operands,
                out_avals=tuple(out_avals),
                in_names=tuple(all_in_names),
                out_names=tuple(out_names),
                lowering_input_output_aliases=(),
                sim_require_finite=True,
                sim_require_nnan=True,
                nc=nc,
            )
            return tuple(outs)

        devices = jax.devices()[:N_CORES]
        mesh = Mesh(np.asarray(devices), ("core",))
        n_params = len(in_names)
        in_specs = (PartitionSpec("core"),) * (n_params + len(out_names))
        out_specs = (PartitionSpec("core"),) * len(out_names)
        self.fn = jax.jit(shard_map(_body, mesh=mesh, in_specs=in_specs,
                                    out_specs=out_specs, check_rep=False),
                          keep_unused=True)

    def __call__(self, in_maps):
        jax = self.jax
        concat_in = [
            np.concatenate([np.asarray(in_maps[c][name]) for c in range(N_CORES)], axis=0)
            for name in self.in_names
        ]
        concat_zeros = [np.zeros((N_CORES * z.shape[0], *z.shape[1:]), z.dtype)
                        for z in self.zero_outs]
        outs = self.fn(*concat_in, *concat_zeros)
        jax.block_until_ready(outs)
        return [
            {name: np.asarray(outs[i]).reshape(N_CORES, *self.out_avals[i].shape)[c]
             for i, name in enumerate(self.out_names)}
            for c in range(N_CORES)
        ]


def make_core_inputs(x, ternary, global_, core, hpc=8):
    n = core // 2
    if hpc == 8:
        heads = list(range(8))
    else:
        hg = core % 2
        heads = list(range(hg * hpc, (hg + 1) * hpc))
    t = ternary[:, :, heads]
    g = global_[:, :, heads]
    return {
        "xT": np.ascontiguousarray(x[n].T.astype(np.float32)),
        "tern_a": np.ascontiguousarray(t.transpose(0, 2, 1).reshape(D, hpc * D).astype(np.float16)),
        "tern_b": np.ascontiguousarray(t.transpose(1, 2, 0).reshape(D, hpc * D).astype(np.float16)),
        "glT": np.ascontiguousarray(g.transpose(1, 2, 0).reshape(D, hpc * G).astype(np.float16)),
        "gl": np.ascontiguousarray(g.transpose(0, 2, 1).reshape(G, hpc * D).astype(np.float16)),
    }


def get_runner(n_iter=4):
    key = ("runner", n_iter)
    if key not in _CACHE:
        nc = build_kernel(n_iter=n_iter, num_devices=N_CORES, hpc=8, use_cc=False)
        _CACHE[key] = _Runner(nc)
    return _CACHE[key]


def kernel(x, mask, ternary, global_):
    x = np.asarray(x, dtype=np.float32)
    mask = np.asarray(mask)
    ternary = np.asarray(ternary, dtype=np.float32)
    global_ = np.asarray(global_, dtype=np.float32)

    run = get_runner(4)
    in_maps = [make_core_inputs(x, ternary, global_, c) for c in range(N_CORES)]
    res = run(in_maps)
    out = np.stack([res[2 * n]["y"] for n in range(B)])
    out = np.where((mask != 0)[..., None], out, np.float32(0.0)).astype(np.float32)
    return out



# revision 16
# speedup vs baseline: 2.8971x; 1.0110x over previous
"""Trainium2 Bass kernel for nn_AbsGlobalHeadProbEncoder (MFVI message passing).

kernel(**inputs) takes the FULL inputs
    x       [4, 1024, 128] f32
    mask    [4, 1024]      i32   (all ones per the problem spec)
    ternary [128, 128, 8]  f32
    global_ [64, 128, 8]   f32
and returns the FULL output [4, 1024, 128] f32.

Sharding: 8 NeuronCores, one batch element per core pair (cores 2n and 2n+1
redundantly compute batch n with all 8 heads). A cross-core AllReduce variant
was measured at ~2.2 ms per 512 KB pairwise all-reduce on this stack - far
more than the ~0.15 ms of per-iteration compute it would save - so full
replication wins.

Per core and MFVI iteration: scores F_H = [H, L, L+G] are built chunk-wise
with fp16 matmuls into fp32 PSUM, softmax'd with a fused exp+rowsum on the
scalar engine, normalized by a per-partition DVE scale, transposed on the PE
(fp16, packed PSUM banks) for the msg_i contraction, and all three messages
accumulate into one shared [D, L] fp32 PSUM region.
"""
import sys
import contextlib

if '/opt/trn_rl_repo' not in sys.path:
    sys.path.insert(0, '/opt/trn_rl_repo')

import numpy as np
import concourse.bacc as bacc
import concourse.mybir as mybir
import concourse.tile as tile
from concourse.masks import make_identity

F32 = mybir.dt.float32
F16 = mybir.dt.float16
AF = mybir.ActivationFunctionType

B = 4
L = 1024
D = 128
H = 8
G = 64
IC = L // 128
N_CORES = 8

_CACHE = {}


def build_kernel(n_iter=4, num_devices=8, groups=None, hpc=4, sb_bufs=2, use_cc=True):
    if groups is None:
        groups = [[2 * i, 2 * i + 1] for i in range(num_devices // 2)]
    nc = bacc.Bacc("TRN2", target_bir_lowering=False, debug=False,
                   num_devices=num_devices)

    xT = nc.declare_dram_parameter("xT", [D, L], F32, isOutput=False)
    tern_a = nc.declare_dram_parameter("tern_a", [D, hpc * D], F16, isOutput=False)
    tern_b = nc.declare_dram_parameter("tern_b", [D, hpc * D], F16, isOutput=False)
    glT = nc.declare_dram_parameter("glT", [D, hpc * G], F16, isOutput=False)
    gl = nc.declare_dram_parameter("gl", [G, hpc * D], F16, isOutput=False)
    y = nc.declare_dram_parameter("y", [L, D], F32, isOutput=True)

    with tile.TileContext(nc) as tc:
        with contextlib.ExitStack() as ctx:
            singles = ctx.enter_context(tc.tile_pool(name="singles", bufs=1))
            sb = ctx.enter_context(tc.tile_pool(name="sb", bufs=sb_bufs))
            hp = ctx.enter_context(tc.tile_pool(name="hp", bufs=2))
            # PSUM budget (8 banks): msgp 2 + fh 2x2 + aux 2x1 = 8
            aux = ctx.enter_context(tc.tile_pool(name="aux", bufs=2, space="PSUM"))
            fh = ctx.enter_context(tc.tile_pool(name="fh", bufs=2, space="PSUM"))
            msgp = ctx.enter_context(tc.tile_pool(name="msgp", bufs=1, space="PSUM"))
            dram = ctx.enter_context(tc.tile_pool(name="dram", bufs=2, space="DRAM"))

            # ---- persistent SBUF state ----
            unaryT = singles.tile([D, L], F32)
            nc.sync.dma_start(unaryT[:], xT[:])
            ta_sb = singles.tile([D, hpc * D], F16)
            nc.sync.dma_start(ta_sb[:], tern_a[:])
            tb_sb = singles.tile([D, hpc * D], F16)
            nc.sync.dma_start(tb_sb[:], tern_b[:])
            glT_sb = singles.tile([D, hpc * G], F16)
            nc.sync.dma_start(glT_sb[:], glT[:])
            gl_sb = singles.tile([G, hpc * D], F16)
            nc.sync.dma_start(gl_sb[:], gl[:])
            id16 = singles.tile([128, 128], F16)
            make_identity(nc, id16[:])
            id32 = singles.tile([128, 128], F32)
            make_identity(nc, id32[:])

            qzT = singles.tile([D, L], F16)
            fzT = singles.tile([D, L], F32)

            def z_tail(fzT_src, last=False):
                """qzT <- softmax_D(fzT_src^T)^T ; if last: y <- fzT_src^T."""
                if last:
                    out_sb = sb.tile([128, L], F32, tag="zout")
                    for ic in range(IC):
                        fz_ps = fh.tile([128, 128], F32, tag="fh_ps")
                        nc.tensor.transpose(fz_ps[:], fzT_src[:, ic * 128:(ic + 1) * 128], id32[:])
                        nc.vector.tensor_copy(out_sb[:, ic * 128:(ic + 1) * 128], fz_ps[:])
                        nc.sync.dma_start(y[ic * 128:(ic + 1) * 128, :],
                                          out_sb[:, ic * 128:(ic + 1) * 128])
                    return
                ez = sb.tile([128, L], F32, tag="ez")
                sums = sb.tile([128, IC], F32, tag="zsums")
                for ic in range(IC):
                    cs = slice(ic * 128, (ic + 1) * 128)
                    fz_ps = fh.tile([128, 128], F32, tag="fh_ps")
                    nc.tensor.transpose(fz_ps[:], fzT_src[:, cs], id32[:])
                    nc.scalar.activation(ez[:, cs], fz_ps[:], AF.Exp,
                                         accum_out=sums[:, ic:ic + 1])
                rz = sb.tile([128, IC], F32, tag="zrz")
                nc.vector.reciprocal(rz[:], sums[:])
                qz_sc = sb.tile([128, L], F16, tag="qzsc")
                for ic in range(IC):
                    cs = slice(ic * 128, (ic + 1) * 128)
                    nc.vector.tensor_scalar_mul(qz_sc[:, cs], ez[:, cs], rz[:, ic:ic + 1])
                qzT_ps = fh.tile([128, L], F16, tag="fh_ps")
                for ic in range(IC):
                    cs = slice(ic * 128, (ic + 1) * 128)
                    nc.tensor.transpose(qzT_ps[:, cs], qz_sc[:, cs], id16[:])
                nc.vector.tensor_copy(qzT[:], qzT_ps[:])

            z_tail(unaryT)

            for it in range(n_iter):
                # ---------- phase A + B, software-pipelined across heads ----------
                # Emission order drives each engine's program order: head h+1's
                # score build (PE) is emitted before head h's message matmuls so
                # the PE streams F(h+1) while ACT/DVE finish exp/normalize(h).
                s_sb = sb.tile([128, IC * hpc * 128], F16, tag="s_sb")
                r_sb = sb.tile([128, IC * hpc * 128], F16, tag="r_sb")
                eg_sb = sb.tile([128, IC * hpc * G], F16, tag="eg_sb")
                eg_sums = sb.tile([128, IC * hpc], F32, tag="eg_sums")
                msg_ps = msgp.tile([128, L], F32, tag="msg_ps")
                sts = [None] * hpc
                ebs = [None] * hpc
                sms = [None] * hpc

                def emit_front(h):
                    hs = slice(h * 128, (h + 1) * 128)
                    st_ps = fh.tile([128, L], F32, tag="fh_ps")
                    for half in range(2):
                        nc.tensor.matmul(st_ps[:, half * 512:(half + 1) * 512],
                                         ta_sb[:, hs], qzT[:, half * 512:(half + 1) * 512])
                    st_sb = hp.tile([128, L], F16, tag="st_sb")
                    nc.scalar.copy(st_sb[:], st_ps[:])
                    e_big = hp.tile([128, IC * L], F16, tag="e_big")
                    sums = hp.tile([128, IC], F32, tag="hsums")
                    for ic in range(IC):
                        fh_ps = fh.tile([128, L], F32, tag="fh_ps")
                        for half in range(2):
                            nc.tensor.matmul(fh_ps[:, half * 512:(half + 1) * 512],
                                             st_sb[:, ic * 128:(ic + 1) * 128],
                                             qzT[:, half * 512:(half + 1) * 512])
                        nc.scalar.activation(e_big[:, ic * L:(ic + 1) * L], fh_ps[:],
                                             AF.Exp, accum_out=sums[:, ic:ic + 1])
                    sts[h], ebs[h], sms[h] = st_sb, e_big, sums

                def emit_phase_a():
                    # s_sb copies on DVE so ACT keeps streaming head-0 exps
                    for c in range(IC):
                        cs = slice(c * 128, (c + 1) * 128)
                        os_ = slice(c * hpc * 128, (c + 1) * hpc * 128)
                        s_ps = fh.tile([128, hpc * 128], F32, tag="fh_ps")
                        for half in range(max(1, hpc * 128 // 512)):
                            nh = slice(half * 512, min((half + 1) * 512, hpc * 128))
                            nc.tensor.matmul(s_ps[:, nh], qzT[:, cs], ta_sb[:, nh])
                        nc.vector.tensor_copy(s_sb[:, os_], s_ps[:])
                    ics_per_bank = max(1, 512 // (hpc * G))
                    for ic2 in range(IC // ics_per_bank):
                        hg_ps = fh.tile([128, ics_per_bank * hpc * G], F32, tag="fh_ps")
                        for k in range(ics_per_bank):
                            ic = ics_per_bank * ic2 + k
                            nc.tensor.matmul(hg_ps[:, k * hpc * G:(k + 1) * hpc * G],
                                             qzT[:, ic * 128:(ic + 1) * 128], glT_sb[:])
                        nc.scalar.activation(
                            eg_sb[:, ic2 * ics_per_bank * hpc * G:(ic2 + 1) * ics_per_bank * hpc * G],
                            hg_ps[:], AF.Exp)
                    nc.vector.reduce_sum(eg_sums[:],
                                         eg_sb.rearrange("p (s g) -> p s g", g=G),
                                         axis=mybir.AxisListType.X)
                    for c in range(IC):
                        cs = slice(c * 128, (c + 1) * 128)
                        os_ = slice(c * hpc * 128, (c + 1) * hpc * 128)
                        r_ps = fh.tile([128, hpc * 128], F32, tag="fh_ps")
                        for half in range(max(1, hpc * 128 // 512)):
                            nh = slice(half * 512, min((half + 1) * 512, hpc * 128))
                            nc.tensor.matmul(r_ps[:, nh], qzT[:, cs], tb_sb[:, nh])
                        nc.scalar.copy(r_sb[:, os_], r_ps[:])

                def emit_back(h):
                    hs = slice(h * 128, (h + 1) * 128)
                    st_sb, e_big, sums = sts[h], ebs[h], sms[h]
                    et_big = hp.tile([128, IC * L], F16, tag="et_big")
                    tot = hp.tile([128, IC], F32, tag="htot")
                    rr = hp.tile([128, IC], F32, tag="hr")
                    eg_h_sums = eg_sums.rearrange("p (s h) -> p s h", h=hpc)[:, :, h]
                    nc.vector.tensor_add(tot[:], sums[:], eg_h_sums)
                    nc.vector.reciprocal(rr[:], tot[:])
                    for ic in range(IC):
                        es = slice(ic * L, (ic + 1) * L)
                        nc.vector.tensor_scalar_mul(e_big[:, es], e_big[:, es],
                                                    rr[:, ic:ic + 1])
                        for half in range(2):
                            nc.tensor.matmul(
                                msg_ps[:, half * 512:(half + 1) * 512],
                                s_sb[:, (ic * hpc + h) * 128:(ic * hpc + h + 1) * 128],
                                e_big[:, ic * L + half * 512: ic * L + (half + 1) * 512],
                                start=(h == 0 and ic == 0), stop=False)
                        t_ps = aux.tile([128, L], F16, tag="aux_ps")
                        for jc in range(IC):
                            nc.tensor.transpose(t_ps[:, jc * 128:(jc + 1) * 128],
                                                e_big[:, ic * L + jc * 128: ic * L + (jc + 1) * 128],
                                                id16[:])
                        cp_eng = nc.vector.tensor_copy
                        cp_eng(
                            et_big.rearrange("p (jc i) -> p jc i", jc=IC)[:, :, ic * 128:(ic + 1) * 128],
                            t_ps.rearrange("p (jc i) -> p jc i", jc=IC))
                    for jc in range(IC):
                        for half in range(2):
                            nc.tensor.matmul(
                                msg_ps[:, half * 512:(half + 1) * 512],
                                r_sb[:, (jc * hpc + h) * 128:(jc * hpc + h + 1) * 128],
                                et_big[:, jc * L + half * 512: jc * L + (half + 1) * 512],
                                start=False, stop=False)
                    egT_ps = aux.tile([64, IC * 128], F16, tag="aux_ps")
                    for ic in range(IC):
                        col = (ic * hpc + h) * G
                        nc.vector.tensor_scalar_mul(eg_sb[:, col:col + G],
                                                    eg_sb[:, col:col + G], rr[:, ic:ic + 1])
                        nc.tensor.transpose(egT_ps[:, ic * 128:(ic + 1) * 128],
                                            eg_sb[:, col:col + G], id16[:])
                    egT_sb = hp.tile([64, IC * 128], F16, tag="egT_sb")
                    nc.scalar.copy(egT_sb[:], egT_ps[:])
                    for half in range(2):
                        nc.tensor.matmul(msg_ps[:, half * 512:(half + 1) * 512],
                                         gl_sb[:, hs],
                                         egT_sb[:, half * 512:(half + 1) * 512],
                                         start=False, stop=(h == hpc - 1))

                emit_front(0)
                emit_phase_a()
                for h in range(hpc):
                    if h + 1 < hpc:
                        emit_front(h + 1)
                    emit_back(h)

                # ---------- phase C: all-reduce + Z update ----------
                if use_cc:
                    msg_sb = sb.tile([128, L], F32, tag="msg_sb")
                    nc.vector.tensor_copy(msg_sb[:], msg_ps[:])
                    bi = dram.tile([128, L], F32, tag="cc_in")
                    bo = dram.tile([128, L], F32, tag="cc_out")
                    nc.sync.dma_start(bi[:], msg_sb[:])
                    nc.gpsimd.collective_compute(
                        "AllReduce", mybir.AluOpType.add,
                        replica_groups=groups,
                        ins=[bi.opt()], outs=[bo.opt()])
                    msg_red = sb.tile([128, L], F32, tag="msg_red")
                    nc.sync.dma_start(msg_red[:], bo[:])
                    nc.vector.tensor_add(fzT[:], msg_red[:], unaryT[:])
                else:
                    nc.vector.tensor_add(fzT[:], msg_ps[:], unaryT[:])
                z_tail(fzT, last=(it == n_iter - 1))

    nc.compile()
    return nc

class _Runner:
    """Keeps the jitted SPMD executable alive across kernel() calls."""

    def __init__(self, nc):
        import jax
        from jax.sharding import Mesh, PartitionSpec
        from jax.experimental.shard_map import shard_map
        from concourse.bass2jax import (_bass_exec_p, install_neuronx_cc_hook,
                                        partition_id_tensor)
        install_neuronx_cc_hook()
        self.jax = jax
        in_names, out_names, out_avals, zero_outs = [], [], [], []
        partition_name = nc.partition_id_tensor.name if nc.partition_id_tensor else None
        for alloc in nc.m.functions[0].allocations:
            if not isinstance(alloc, mybir.MemoryLocationSet):
                continue
            name = alloc.memorylocations[0].name
            if alloc.kind == "ExternalInput":
                if name != partition_name:
                    in_names.append(name)
            elif alloc.kind == "ExternalOutput":
                out_names.append(name)
                shape = tuple(alloc.tensor_shape)
                dtype = mybir.dt.np(alloc.dtype)
                out_avals.append(jax.core.ShapedArray(shape, dtype))
                zero_outs.append(np.zeros(shape, dtype))
        self.in_names, self.out_names = in_names, out_names
        self.out_avals, self.zero_outs = out_avals, zero_outs
        all_in_names = list(in_names) + list(out_names)
        if partition_name is not None:
            all_in_names.append(partition_name)

        def _body(*args):
            operands = list(args)
            if partition_name is not None:
                operands.append(partition_id_tensor())
            outs = _bass_exec_p.bind(
                *operands,
                out_avals=tuple(out_avals),
                in_names=tuple(all_in_names),
                out_names=tuple(out_names),
                lowering_input_output_aliases=(),
                sim_require_finite=True,
                sim_require_nnan=True,
                nc=nc,
            )
            return tuple(outs)

        devices = jax.devices()[:N_CORES]
        mesh = Mesh(np.asarray(devices), ("core",))
        n_params = len(in_names)
        in_specs = (PartitionSpec("core"),) * (n_params + len(out_names))
        out_specs = (PartitionSpec("core"),) * len(out_names)
        self.fn = jax.jit(shard_map(_body, mesh=mesh, in_specs=in_specs,
                                    out_specs=out_specs, check_rep=False),
                          keep_unused=True)

    def __call__(self, in_maps):
        jax = self.jax
        concat_in = [
            np.concatenate([np.asarray(in_maps[c][name]) for c in range(N_CORES)], axis=0)
            for name in self.in_names
        ]
        concat_zeros = [np.zeros((N_CORES * z.shape[0], *z.shape[1:]), z.dtype)
                        for z in self.zero_outs]
        outs = self.fn(*concat_in, *concat_zeros)
        jax.block_until_ready(outs)
        return [
            {name: np.asarray(outs[i]).reshape(N_CORES, *self.out_avals[i].shape)[c]
             for i, name in enumerate(self.out_names)}
            for c in range(N_CORES)
        ]


def make_core_inputs(x, ternary, global_, core, hpc=8):
    n = core // 2
    if hpc == 8:
        heads = list(range(8))
    else:
        hg = core % 2
        heads = list(range(hg * hpc, (hg + 1) * hpc))
    t = ternary[:, :, heads]
    g = global_[:, :, heads]
    return {
        "xT": np.ascontiguousarray(x[n].T.astype(np.float32)),
        "tern_a": np.ascontiguousarray(t.transpose(0, 2, 1).reshape(D, hpc * D).astype(np.float16)),
        "tern_b": np.ascontiguousarray(t.transpose(1, 2, 0).reshape(D, hpc * D).astype(np.float16)),
        "glT": np.ascontiguousarray(g.transpose(1, 2, 0).reshape(D, hpc * G).astype(np.float16)),
        "gl": np.ascontiguousarray(g.transpose(0, 2, 1).reshape(G, hpc * D).astype(np.float16)),
    }


def get_runner(n_iter=4):
    key = ("runner", n_iter)
    if key not in _CACHE:
        nc = build_kernel(n_iter=n_iter, num_devices=N_CORES, hpc=8, use_cc=False)
        _CACHE[key] = _Runner(nc)
    return _CACHE[key]


def kernel(x, mask, ternary, global_):
    x = np.asarray(x, dtype=np.float32)
    mask = np.asarray(mask)
    ternary = np.asarray(ternary, dtype=np.float32)
    global_ = np.asarray(global_, dtype=np.float32)

    run = get_runner(4)
    in_maps = [make_core_inputs(x, ternary, global_, c) for c in range(N_CORES)]
    res = run(in_maps)
    out = np.stack([res[2 * n]["y"] for n in range(B)])
    out = np.where((mask != 0)[..., None], out, np.float32(0.0)).astype(np.float32)
    return out



# revision 17
# speedup vs baseline: 3.9824x; 1.3746x over previous
"""Trainium2 Bass kernel for nn_AbsGlobalHeadProbEncoder (MFVI message passing).

kernel(**inputs) takes the FULL inputs
    x       [4, 1024, 128] f32
    mask    [4, 1024]      i32   (all ones per the problem spec)
    ternary [128, 128, 8]  f32
    global_ [64, 128, 8]   f32
and returns the FULL output [4, 1024, 128] f32.

Sharding: 8 NeuronCores, one batch element per core pair (cores 2n and 2n+1
redundantly compute batch n with all 8 heads). A cross-core AllReduce variant
was measured at ~2.2 ms per 512 KB pairwise all-reduce on this stack - far
more than the ~0.15 ms of per-iteration compute it would save - so full
replication wins.

Per core and MFVI iteration: scores F_H = [H, L, L+G] are built chunk-wise
with fp16 matmuls into fp32 PSUM, softmax'd with a fused exp+rowsum on the
scalar engine, normalized by a per-partition DVE scale, transposed on the PE
(fp16, packed PSUM banks) for the msg_i contraction, and all three messages
accumulate into one shared [D, L] fp32 PSUM region.
"""
import sys
import contextlib

if '/opt/trn_rl_repo' not in sys.path:
    sys.path.insert(0, '/opt/trn_rl_repo')

import numpy as np
import concourse.bacc as bacc
import concourse.mybir as mybir
import concourse.tile as tile
from concourse.masks import make_identity

F32 = mybir.dt.float32
F16 = mybir.dt.float16
AF = mybir.ActivationFunctionType

B = 4
L = 1024
D = 128
H = 8
G = 64
IC = L // 128
N_CORES = 8

_CACHE = {}


def build_kernel(n_iter=4, num_devices=8, groups=None, hpc=4, sb_bufs=2, use_cc=True):
    if groups is None:
        groups = [[2 * i, 2 * i + 1] for i in range(num_devices // 2)]
    nc = bacc.Bacc("TRN2", target_bir_lowering=False, debug=False,
                   num_devices=num_devices)

    xT = nc.declare_dram_parameter("xT", [D, L], F32, isOutput=False)
    tern_a = nc.declare_dram_parameter("tern_a", [D, hpc * D], F16, isOutput=False)
    tern_b = nc.declare_dram_parameter("tern_b", [D, hpc * D], F16, isOutput=False)
    glT = nc.declare_dram_parameter("glT", [D, hpc * G], F16, isOutput=False)
    gl2 = nc.declare_dram_parameter("gl2", [2 * G, (hpc // 2) * D], F16, isOutput=False)
    y = nc.declare_dram_parameter("y", [L, D], F32, isOutput=True)

    with tile.TileContext(nc) as tc:
        with contextlib.ExitStack() as ctx:
            singles = ctx.enter_context(tc.tile_pool(name="singles", bufs=1))
            sb = ctx.enter_context(tc.tile_pool(name="sb", bufs=sb_bufs))
            hp = ctx.enter_context(tc.tile_pool(name="hp", bufs=2))
            # PSUM budget (8 banks): msgp 2 + fh 2x2 + aux 2x1 = 8
            aux = ctx.enter_context(tc.tile_pool(name="aux", bufs=2, space="PSUM"))
            fh = ctx.enter_context(tc.tile_pool(name="fh", bufs=2, space="PSUM"))
            msgp = ctx.enter_context(tc.tile_pool(name="msgp", bufs=1, space="PSUM"))
            dram = ctx.enter_context(tc.tile_pool(name="dram", bufs=2, space="DRAM"))

            # ---- persistent SBUF state ----
            unaryT = singles.tile([D, L], F32)
            nc.sync.dma_start(unaryT[:], xT[:])
            ta_sb = singles.tile([D, hpc * D], F16)
            nc.sync.dma_start(ta_sb[:], tern_a[:])
            tb_sb = singles.tile([D, hpc * D], F16)
            nc.sync.dma_start(tb_sb[:], tern_b[:])
            glT_sb = singles.tile([D, hpc * G], F16)
            nc.sync.dma_start(glT_sb[:], glT[:])
            gl2_sb = singles.tile([2 * G, (hpc // 2) * D], F16)
            nc.sync.dma_start(gl2_sb[:], gl2[:])
            id16 = singles.tile([128, 128], F16)
            make_identity(nc, id16[:])
            id32 = singles.tile([128, 128], F32)
            make_identity(nc, id32[:])

            qzT = singles.tile([D, L], F16)
            fzT = singles.tile([D, L], F32)

            def z_tail(fzT_src, last=False):
                """qzT <- softmax_D(fzT_src^T)^T ; if last: y <- fzT_src^T."""
                if last:
                    out_sb = sb.tile([128, L], F32, tag="zout")
                    for ic in range(IC):
                        fz_ps = fh.tile([128, 128], F32, tag="fh_ps")
                        nc.tensor.transpose(fz_ps[:], fzT_src[:, ic * 128:(ic + 1) * 128], id32[:])
                        nc.vector.tensor_copy(out_sb[:, ic * 128:(ic + 1) * 128], fz_ps[:])
                        nc.sync.dma_start(y[ic * 128:(ic + 1) * 128, :],
                                          out_sb[:, ic * 128:(ic + 1) * 128])
                    return
                ez = sb.tile([128, L], F32, tag="ez")
                sums = sb.tile([128, IC], F32, tag="zsums")
                for ic in range(IC):
                    cs = slice(ic * 128, (ic + 1) * 128)
                    fz_ps = fh.tile([128, 128], F32, tag="fh_ps")
                    nc.tensor.transpose(fz_ps[:], fzT_src[:, cs], id32[:])
                    nc.scalar.activation(ez[:, cs], fz_ps[:], AF.Exp,
                                         accum_out=sums[:, ic:ic + 1])
                rz = sb.tile([128, IC], F32, tag="zrz")
                nc.vector.reciprocal(rz[:], sums[:])
                qz_sc = sb.tile([128, L], F16, tag="qzsc")
                for ic in range(IC):
                    cs = slice(ic * 128, (ic + 1) * 128)
                    nc.vector.tensor_scalar_mul(qz_sc[:, cs], ez[:, cs], rz[:, ic:ic + 1])
                qzT_ps = fh.tile([128, L], F16, tag="fh_ps")
                for ic in range(IC):
                    cs = slice(ic * 128, (ic + 1) * 128)
                    nc.tensor.transpose(qzT_ps[:, cs], qz_sc[:, cs], id16[:])
                nc.vector.tensor_copy(qzT[:], qzT_ps[:])

            z_tail(unaryT)

            for it in range(n_iter):
                # ---------- phase A + B, software-pipelined across heads ----------
                # Emission order drives each engine's program order: head h+1's
                # score build (PE) is emitted before head h's message matmuls so
                # the PE streams F(h+1) while ACT/DVE finish exp/normalize(h).
                s_sb = sb.tile([128, IC * hpc * 128], F16, tag="s_sb")
                r_sb = sb.tile([128, IC * hpc * 128], F16, tag="r_sb")
                eg_sb = sb.tile([128, IC * hpc * G], F16, tag="eg_sb")
                eg_sums = sb.tile([128, IC * hpc], F32, tag="eg_sums")
                msg_ps = msgp.tile([128, L], F32, tag="msg_ps")
                sts = [None] * hpc
                ebs = [None] * hpc
                sms = [None] * hpc

                def emit_front(h):
                    hs = slice(h * 128, (h + 1) * 128)
                    st_ps = fh.tile([128, L], F32, tag="fh_ps")
                    for half in range(2):
                        nc.tensor.matmul(st_ps[:, half * 512:(half + 1) * 512],
                                         ta_sb[:, hs], qzT[:, half * 512:(half + 1) * 512])
                    st_sb = hp.tile([128, L], F16, tag="st_sb")
                    nc.scalar.copy(st_sb[:], st_ps[:])
                    e_big = hp.tile([128, IC * L], F16, tag="e_big")
                    sums = hp.tile([128, IC], F32, tag="hsums")
                    for ic in range(IC):
                        fh_ps = fh.tile([128, L], F32, tag="fh_ps")
                        for half in range(2):
                            nc.tensor.matmul(fh_ps[:, half * 512:(half + 1) * 512],
                                             st_sb[:, ic * 128:(ic + 1) * 128],
                                             qzT[:, half * 512:(half + 1) * 512])
                        nc.scalar.activation(e_big[:, ic * L:(ic + 1) * L], fh_ps[:],
                                             AF.Exp, accum_out=sums[:, ic:ic + 1])
                    sts[h], ebs[h], sms[h] = st_sb, e_big, sums

                def emit_phase_a():
                    # s_sb copies on DVE so ACT keeps streaming head-0 exps
                    for c in range(IC):
                        cs = slice(c * 128, (c + 1) * 128)
                        os_ = slice(c * hpc * 128, (c + 1) * hpc * 128)
                        s_ps = fh.tile([128, hpc * 128], F32, tag="fh_ps")
                        for half in range(max(1, hpc * 128 // 512)):
                            nh = slice(half * 512, min((half + 1) * 512, hpc * 128))
                            nc.tensor.matmul(s_ps[:, nh], qzT[:, cs], ta_sb[:, nh])
                        nc.vector.tensor_copy(s_sb[:, os_], s_ps[:])
                    ics_per_bank = max(1, 512 // (hpc * G))
                    for ic2 in range(IC // ics_per_bank):
                        hg_ps = fh.tile([128, ics_per_bank * hpc * G], F32, tag="fh_ps")
                        for k in range(ics_per_bank):
                            ic = ics_per_bank * ic2 + k
                            nc.tensor.matmul(hg_ps[:, k * hpc * G:(k + 1) * hpc * G],
                                             qzT[:, ic * 128:(ic + 1) * 128], glT_sb[:])
                        nc.scalar.activation(
                            eg_sb[:, ic2 * ics_per_bank * hpc * G:(ic2 + 1) * ics_per_bank * hpc * G],
                            hg_ps[:], AF.Exp)
                    nc.vector.reduce_sum(eg_sums[:],
                                         eg_sb.rearrange("p (s g) -> p s g", g=G),
                                         axis=mybir.AxisListType.X)
                    for c in range(IC):
                        cs = slice(c * 128, (c + 1) * 128)
                        os_ = slice(c * hpc * 128, (c + 1) * hpc * 128)
                        r_ps = fh.tile([128, hpc * 128], F32, tag="fh_ps")
                        for half in range(max(1, hpc * 128 // 512)):
                            nh = slice(half * 512, min((half + 1) * 512, hpc * 128))
                            nc.tensor.matmul(r_ps[:, nh], qzT[:, cs], tb_sb[:, nh])
                        nc.scalar.copy(r_sb[:, os_], r_ps[:])

                def emit_back(h):
                    hs = slice(h * 128, (h + 1) * 128)
                    st_sb, e_big, sums = sts[h], ebs[h], sms[h]
                    et_big = hp.tile([128, IC * L], F16, tag="et_big")
                    tot = hp.tile([128, IC], F32, tag="htot")
                    rr = hp.tile([128, IC], F32, tag="hr")
                    eg_h_sums = eg_sums.rearrange("p (s h) -> p s h", h=hpc)[:, :, h]
                    nc.vector.tensor_add(tot[:], sums[:], eg_h_sums)
                    nc.vector.reciprocal(rr[:], tot[:])
                    for ic in range(IC):
                        es = slice(ic * L, (ic + 1) * L)
                        nc.vector.tensor_scalar_mul(e_big[:, es], e_big[:, es],
                                                    rr[:, ic:ic + 1])
                        for half in range(2):
                            nc.tensor.matmul(
                                msg_ps[:, half * 512:(half + 1) * 512],
                                s_sb[:, (ic * hpc + h) * 128:(ic * hpc + h + 1) * 128],
                                e_big[:, ic * L + half * 512: ic * L + (half + 1) * 512],
                                start=(h == 0 and ic == 0), stop=False)
                        t_ps = aux.tile([128, L], F16, tag="aux_ps")
                        for jc in range(IC):
                            nc.tensor.transpose(t_ps[:, jc * 128:(jc + 1) * 128],
                                                e_big[:, ic * L + jc * 128: ic * L + (jc + 1) * 128],
                                                id16[:])
                        cp_eng = nc.vector.tensor_copy
                        cp_eng(
                            et_big.rearrange("p (jc i) -> p jc i", jc=IC)[:, :, ic * 128:(ic + 1) * 128],
                            t_ps.rearrange("p (jc i) -> p jc i", jc=IC))
                    for jc in range(IC):
                        for half in range(2):
                            nc.tensor.matmul(
                                msg_ps[:, half * 512:(half + 1) * 512],
                                r_sb[:, (jc * hpc + h) * 128:(jc * hpc + h + 1) * 128],
                                et_big[:, jc * L + half * 512: jc * L + (half + 1) * 512],
                                start=False, stop=False)
                    for ic in range(IC):
                        col = (ic * hpc + h) * G
                        nc.vector.tensor_scalar_mul(eg_sb[:, col:col + G],
                                                    eg_sb[:, col:col + G], rr[:, ic:ic + 1])

                emit_front(0)
                emit_phase_a()
                for h in range(hpc):
                    if h + 1 < hpc:
                        emit_front(h + 1)
                    emit_back(h)

                # eg tail: head-pair-stacked transposes + 2G-deep msg_g matmuls
                for k in range(hpc // 2):
                    egT_ps = aux.tile([128, IC * 128], F16, tag="aux_ps")
                    for ic in range(IC):
                        col = (ic * hpc + 2 * k) * G
                        nc.tensor.transpose(egT_ps[:, ic * 128:(ic + 1) * 128],
                                            eg_sb[:, col:col + 2 * G], id16[:])
                    egT_sb = hp.tile([128, IC * 128], F16, tag="egT_sb")
                    nc.scalar.copy(egT_sb[:], egT_ps[:])
                    for half in range(2):
                        nc.tensor.matmul(msg_ps[:, half * 512:(half + 1) * 512],
                                         gl2_sb[:, k * 128:(k + 1) * 128],
                                         egT_sb[:, half * 512:(half + 1) * 512],
                                         start=False, stop=(k == hpc // 2 - 1))

                # ---------- phase C: all-reduce + Z update ----------
                if use_cc:
                    msg_sb = sb.tile([128, L], F32, tag="msg_sb")
                    nc.vector.tensor_copy(msg_sb[:], msg_ps[:])
                    bi = dram.tile([128, L], F32, tag="cc_in")
                    bo = dram.tile([128, L], F32, tag="cc_out")
                    nc.sync.dma_start(bi[:], msg_sb[:])
                    nc.gpsimd.collective_compute(
                        "AllReduce", mybir.AluOpType.add,
                        replica_groups=groups,
                        ins=[bi.opt()], outs=[bo.opt()])
                    msg_red = sb.tile([128, L], F32, tag="msg_red")
                    nc.sync.dma_start(msg_red[:], bo[:])
                    nc.vector.tensor_add(fzT[:], msg_red[:], unaryT[:])
                else:
                    nc.vector.tensor_add(fzT[:], msg_ps[:], unaryT[:])
                z_tail(fzT, last=(it == n_iter - 1))

    nc.compile()
    return nc

class _Runner:
    """Keeps the jitted SPMD executable alive across kernel() calls."""

    def __init__(self, nc):
        import jax
        from jax.sharding import Mesh, PartitionSpec
        from jax.experimental.shard_map import shard_map
        from concourse.bass2jax import (_bass_exec_p, install_neuronx_cc_hook,
                                        partition_id_tensor)
        install_neuronx_cc_hook()
        self.jax = jax
        in_names, out_names, out_avals, zero_outs = [], [], [], []
        partition_name = nc.partition_id_tensor.name if nc.partition_id_tensor else None
        for alloc in nc.m.functions[0].allocations:
            if not isinstance(alloc, mybir.MemoryLocationSet):
                continue
            name = alloc.memorylocations[0].name
            if alloc.kind == "ExternalInput":
                if name != partition_name:
                    in_names.append(name)
            elif alloc.kind == "ExternalOutput":
                out_names.append(name)
                shape = tuple(alloc.tensor_shape)
                dtype = mybir.dt.np(alloc.dtype)
                out_avals.append(jax.core.ShapedArray(shape, dtype))
                zero_outs.append(np.zeros(shape, dtype))
        self.in_names, self.out_names = in_names, out_names
        self.out_avals, self.zero_outs = out_avals, zero_outs
        all_in_names = list(in_names) + list(out_names)
        if partition_name is not None:
            all_in_names.append(partition_name)

        def _body(*args):
            operands = list(args)
            if partition_name is not None:
                operands.append(partition_id_tensor())
            outs = _bass_exec_p.bind(
                *operands,
                out_avals=tuple(out_avals),
                in_names=tuple(all_in_names),
                out_names=tuple(out_names),
                lowering_input_output_aliases=(),
                sim_require_finite=True,
                sim_require_nnan=True,
                nc=nc,
            )
            return tuple(outs)

        devices = jax.devices()[:N_CORES]
        mesh = Mesh(np.asarray(devices), ("core",))
        n_params = len(in_names)
        in_specs = (PartitionSpec("core"),) * (n_params + len(out_names))
        out_specs = (PartitionSpec("core"),) * len(out_names)
        self.fn = jax.jit(shard_map(_body, mesh=mesh, in_specs=in_specs,
                                    out_specs=out_specs, check_rep=False),
                          keep_unused=True)

    def __call__(self, in_maps):
        jax = self.jax
        concat_in = [
            np.concatenate([np.asarray(in_maps[c][name]) for c in range(N_CORES)], axis=0)
            for name in self.in_names
        ]
        concat_zeros = [np.zeros((N_CORES * z.shape[0], *z.shape[1:]), z.dtype)
                        for z in self.zero_outs]
        outs = self.fn(*concat_in, *concat_zeros)
        jax.block_until_ready(outs)
        return [
            {name: np.asarray(outs[i]).reshape(N_CORES, *self.out_avals[i].shape)[c]
             for i, name in enumerate(self.out_names)}
            for c in range(N_CORES)
        ]


def make_core_inputs(x, ternary, global_, core, hpc=8):
    n = core // 2
    if hpc == 8:
        heads = list(range(8))
    else:
        hg = core % 2
        heads = list(range(hg * hpc, (hg + 1) * hpc))
    t = ternary[:, :, heads]
    g = global_[:, :, heads]
    return {
        "xT": np.ascontiguousarray(x[n].T.astype(np.float32)),
        "tern_a": np.ascontiguousarray(t.transpose(0, 2, 1).reshape(D, hpc * D).astype(np.float16)),
        "tern_b": np.ascontiguousarray(t.transpose(1, 2, 0).reshape(D, hpc * D).astype(np.float16)),
        "glT": np.ascontiguousarray(g.transpose(1, 2, 0).reshape(D, hpc * G).astype(np.float16)),
        "gl2": np.ascontiguousarray(
            g.transpose(2, 0, 1).reshape(hpc // 2, 2, G, D)
             .transpose(1, 2, 0, 3).reshape(2 * G, (hpc // 2) * D).astype(np.float16)),
    }


def get_runner(n_iter=4):
    key = ("runner", n_iter)
    if key not in _CACHE:
        nc = build_kernel(n_iter=n_iter, num_devices=N_CORES, hpc=8, use_cc=False)
        _CACHE[key] = _Runner(nc)
    return _CACHE[key]


def kernel(x, mask, ternary, global_):
    x = np.asarray(x, dtype=np.float32)
    mask = np.asarray(mask)
    ternary = np.asarray(ternary, dtype=np.float32)
    global_ = np.asarray(global_, dtype=np.float32)

    run = get_runner(4)
    in_maps = [make_core_inputs(x, ternary, global_, c) for c in range(N_CORES)]
    res = run(in_maps)
    out = np.stack([res[2 * n]["y"] for n in range(B)])
    out = np.where((mask != 0)[..., None], out, np.float32(0.0)).astype(np.float32)
    return out

